# revision 29
# baseline (speedup 1.0000x reference)
"""GatedLTMMemory kernel for 8 Trainium2 NeuronCores.

Data-parallel over the 4096 flattened (B,N) tokens: 512 tokens per core.
Memory-slot tables and weights are replicated. The reference's per-selected-slot
projections (137 GFLOP) are replaced by projecting the slot tables once and
running a masked full-softmax over all S slots (exactly equivalent math).

Precision plan (fp32 matmuls run at 1/4 PE rate; float32r/bf16 at full rate):
  exact fp32 : selection path (q projection, slot norms, scores) — the top-32
               boundary gaps are ~1e-6 so this path cannot be rounded.
  float32r   : Kp/Vp/qh projections, attention logits, Wo/Wout epilogue
               (~1.6e-4 measured on HW).
  bf16       : softmax weights w = exp(att)*mask and the value table Vp
               (~2e-3; the denominators come from the same w so it cancels).

Emission order is chosen so the DVE top-k overlaps the PE Kp/Vp/qh
projections. SBUF pool tags are allocated statically, so dead tensors donate
their slots to later tensors (chains are noted inline). Host passes
weights/tables pre-transposed (layout prep only; no FLOPs moved to host).
"""

import numpy as np

import concourse.bacc as bacc
import concourse.mybir as mybir
import concourse.tile as tile
from concourse.bass import ds, ts
from concourse.bass_utils import run_bass_kernel_spmd
from concourse.masks import make_identity

B, N, QD, D, S, H, K = 4, 1024, 320, 512, 1024, 8, 32
DH = D // H
EPS = 1e-5
P = 128
T = 512                       # tokens per core
NCORES = 8
NT = T // P                   # 4 token tiles
ND = D // P                   # 4 contraction chunks over D
NS = S // P                   # 8 slot tiles
NEG = -1e30
QD_TILES = [(0, 128), (128, 128), (256, 64)]

f32 = mybir.dt.float32
f32r = mybir.dt.float32r
bf16 = mybir.dt.bfloat16
AF = mybir.ActivationFunctionType
OP = mybir.AluOpType

_CACHE: dict = {}


def _build_nc():
    nc = bacc.Bacc("TRN2", target_bir_lowering=False, debug=False)

    dr = {}

    def din(name, shape, dt_):
        dr[name] = nc.dram_tensor(name, shape, dt_, kind="ExternalInput")

    din("queryT", (QD, T), f32)
    din("WqpT", (QD, D), f32)
    din("WqT", (D, D), f32r)
    din("WkT", (D, D), f32r)
    din("WvT", (D, D), f32r)
    din("WoT", (D, D), f32r)
    din("WoutT", (D, QD), f32r)
    din("memkT", (D, S), f32)
    din("memvT", (D, S), f32)
    din("ln_g", (D,), f32)
    din("ln_b", (D,), f32)
    din("bout", (384,), f32)
    out_dram = nc.dram_tensor("outT", (QD, T), f32, kind="ExternalOutput")

    with tile.TileContext(nc) as tc:
        with (
            tc.tile_pool(name="const", bufs=1) as const,
            tc.tile_pool(name="main", bufs=1) as main,
            tc.tile_pool(name="scr2", bufs=2) as scr2,
            tc.tile_pool(name="scr4", bufs=6) as scr4,
            tc.tile_pool(name="psA", bufs=2, space="PSUM") as psA,
            tc.tile_pool(name="psB", bufs=1, space="PSUM") as psB,
            tc.tile_pool(name="psmm", bufs=2, space="PSUM") as psmm,
            nc.allow_low_precision(reason="validated f32r/bf16 paths"),
        ):
            # ---------- constants ----------
            ident = const.tile([P, P], bf16, tag="ident")
            make_identity(nc, ident)
            ones_col = const.tile([P, 1], f32, tag="ones_col")
            nc.vector.memset(ones_col, 1.0)
            ones_row = const.tile([1, P], f32, tag="ones_row")
            nc.vector.memset(ones_row, 1.0)
            # f32r half-ones rows for per-head-pair broadcast matmuls
            halfsel = const.tile([1, 2 * P], f32, tag="halfsel")
            nc.vector.memset(halfsel, 0.0)
            nc.vector.memset(halfsel[0:1, 64:192], 1.0)
            halfsel_r = const.tile([1, 2 * P], f32r, tag="halfsel_r")
            nc.scalar.copy(halfsel_r[:], halfsel[:])
            # halfsel layout: [0:64]=0, [64:192]=1, [192:256]=0
            ones_row_r = halfsel_r[0:1, 64:192]  # [1,128] all ones
            selA = halfsel_r[0:1, 128:256]       # [1,128]: ones x64, zeros x64
            selB = halfsel_r[0:1, 0:128]         # [1,128]: zeros x64, ones x64
            eps_tab = const.tile([1, 1], f32, tag="eps_tab")
            nc.vector.memset(eps_tab, 1e-12)
            eps_ln = const.tile([1, 1], f32, tag="eps_ln")
            nc.vector.memset(eps_ln, EPS)

            # ---------- weight loads ----------
            def load_rows(name, cols, row_tiles, tags, dt_):
                tiles = []
                for (off, sz), tag in zip(row_tiles, tags):
                    t_ = main.tile([sz, cols], dt_, tag=tag, name=f"ld_{tag}")
                    nc.sync.dma_start(t_[:], dr[name].ap()[ds(off, sz), :])
                    tiles.append(t_)
                return tiles

            d_rows = [(i * P, P) for i in range(ND)]
            qryT = load_rows("queryT", T, QD_TILES, ["qry0", "qry1", "qry2"], f32)
            wqpT = load_rows("WqpT", D, QD_TILES, ["wqp0", "wqp1", "wqp2"], f32)
            wqT = load_rows("WqT", D, d_rows, [f"wq{i}" for i in range(ND)], f32r)
            wkT = load_rows("WkT", D, d_rows, [f"wkw{i}" for i in range(ND)], f32r)
            wvT = load_rows("WvT", D, d_rows, [f"wvw{i}" for i in range(ND)], f32r)
            woT = load_rows("WoT", D, d_rows, [f"wo{i}" for i in range(ND)], f32r)
            woutT = load_rows("WoutT", QD, d_rows, [f"wu{i}" for i in range(ND)], f32r)

            g_sb = const.tile([P, ND], f32, tag="g")
            nc.sync.dma_start(g_sb[:], dr["ln_g"].ap().rearrange("(o p) -> p o", p=P))
            b_sb = const.tile([P, ND], f32, tag="b")
            nc.sync.dma_start(b_sb[:], dr["ln_b"].ap().rearrange("(o p) -> p o", p=P))
            bout_sb = const.tile([P, 3], f32, tag="bout")
            nc.sync.dma_start(bout_sb[:], dr["bout"].ap().rearrange("(o p) -> p o", p=P))

            # ---------- slot tables: l2-normalize in transposed layout ----------
            def normalize_table(name, tags):
                tiles = []
                for i in range(ND):
                    t_ = main.tile([P, S], f32, tag=tags[i], name=f"tb_{tags[i]}")
                    nc.sync.dma_start(t_[:], dr[name].ap()[ds(i * P, P), :])
                    tiles.append(t_)
                # sum over d (partitions) of x^2, via ACT square + fp32 ones matmul
                ps_halves = []
                for half in range(2):
                    if half == 0:
                        ps_ssq = psB.tile([1, T], f32, tag="row", name="ssq0")
                    else:
                        ps_ssq = psA.tile([1, T], f32, tag="ctx", name="ssq1")
                    for i in range(ND):
                        sq = scr2.tile([P, T], f32, tag="sq")
                        nc.scalar.square(sq, tiles[i][:, ds(half * T, T)])
                        nc.tensor.matmul(
                            ps_ssq, lhsT=ones_col, rhs=sq,
                            start=(i == 0), stop=(i == ND - 1),
                        )
                    ps_halves.append(ps_ssq)
                sd_row = main.tile([1, S], f32, tag="sdrow", name="sdr")
                for half in range(2):
                    nc.scalar.activation(
                        sd_row[:, ds(half * T, T)], ps_halves[half], AF.Sqrt,
                        bias=eps_tab[:],
                    )
                rsq_row = main.tile([1, S], f32, tag="rsqrow", name="rsq")
                nc.vector.reciprocal(rsq_row, sd_row)
                rsqB = main.tile([P, S], f32, tag="rsqB", name="rsqB")
                for half in range(2):
                    ps_b = psB.tile([P, T], f32, tag="bc")
                    nc.tensor.matmul(
                        ps_b, lhsT=ones_row, rhs=rsq_row[:, ds(half * T, T)],
                        start=True, stop=True,
                    )
                    nc.scalar.copy(rsqB[:, ds(half * T, T)], ps_b)
                for i in range(ND):
                    nc.vector.tensor_tensor(tiles[i][:], tiles[i][:], rsqB[:], OP.mult)
                return tiles

            # keys first; t14 slots chain: keysnT -> topk scratch -> mask01
            keysnT = normalize_table("memkT", [f"t14_{i}" for i in range(ND)])
            # rounded copy of keysnT for the f32r KpT matmul (scores keep fp32)
            ktr = []
            for i in range(ND):
                t_ = main.tile([P, S], f32r, tag=f"ktr{i}", name=f"ktr{i}")
                nc.vector.tensor_copy(t_[:], keysnT[i][:])
                ktr.append(t_)
            # vals; t58 slots chain: valsnT -> scores
            valsnT = normalize_table("memvT", [f"t58_{i}" for i in range(ND)])
            vtr_tags = ["sdrow", "rsqrow", "rsqB", "vtr3"]
            vtr = []
            for i in range(ND):
                t_ = main.tile([P, S], f32r, tag=vtr_tags[i], name=f"vtr{i}")
                nc.vector.tensor_copy(t_[:], valsnT[i][:])
                vtr.append(t_)

            # ---------- qT[d, t] = Wqp @ query.T (exact fp32; f32r copy for qh) ----
            qTr_tags = ["qry0", "qry1", "qry2", "wqp0"]
            qT = []
            for dt_i in range(ND):
                t_ = main.tile([P, T], f32, tag=f"qt{dt_i}", name=f"q{dt_i}")
                ps = psmm.tile([P, T], f32, tag="mm")
                for c in range(3):
                    nc.tensor.matmul(
                        ps, lhsT=wqpT[c][:, ts(dt_i, P)], rhs=qryT[c][:],
                        start=(c == 0), stop=(c == 2),
                    )
                nc.scalar.copy(t_[:], ps)
                qT.append(t_)
            qTr = []
            for dt_i in range(ND):
                tr_ = main.tile([P, T], f32r, tag=qTr_tags[dt_i], name=f"qr{dt_i}")
                nc.vector.tensor_copy(tr_[:], qT[dt_i][:])
                qTr.append(tr_)

            # ---------- scores[t, s] = q @ keysn.T (exact fp32), then top-32 ------
            sc = []
            for tt in range(NT):
                t_ = main.tile([P, S], f32, tag=f"t58_{tt}", name=f"sc{tt}")
                for half in range(2):
                    ps = psmm.tile([P, T], f32, tag="mm")
                    for dc in range(ND):
                        nc.tensor.matmul(
                            ps,
                            lhsT=qT[dc][:, ts(tt, P)],
                            rhs=keysnT[dc][:, ds(half * T, T)],
                            start=(dc == 0), stop=(dc == ND - 1),
                        )
                    nc.scalar.copy(t_[:, ds(half * T, T)], ps)
                sc.append(t_)

            # top-32 threshold per token row (4 rounds of max8), then bf16 mask
            mask01 = []
            for tt in range(NT):
                work = main.tile([P, S], f32, tag=f"t14_{tt}", name=f"wk{tt}")
                cur = sc[tt]
                for r in range(4):
                    mx = main.tile([P, 8], f32, tag=f"mx{tt}_{r}", name=f"mx{tt}_{r}")
                    nc.vector.max(out=mx[:], in_=cur[:])
                    if r < 3:
                        nc.vector.match_replace(
                            out=work[:], in_to_replace=mx[:], in_values=cur[:],
                            imm_value=NEG,
                        )
                        cur = work
                m_ = main.tile([P, S], bf16, tag=f"t14_{tt}", name=f"mk{tt}")
                nc.vector.tensor_scalar(
                    m_[:], sc[tt][:], mx[:, 7:8], None, op0=OP.is_ge
                )
                mask01.append(m_)

            # ---------- KpT[e, s] = Wk @ keysn.T  (f32r) ----------
            kpT = []
            for e in range(ND):
                t_ = main.tile([P, S], f32r, tag=f"kp{e}", name=f"kp{e}")
                for half in range(2):
                    ps = psmm.tile([P, T], f32, tag="mm")
                    for dc in range(ND):
                        nc.tensor.matmul(
                            ps,
                            lhsT=wkT[dc][:, ts(e, P)],
                            rhs=ktr[dc][:, ds(half * T, T)],
                            start=(dc == 0), stop=(dc == ND - 1),
                        )
                    nc.scalar.copy(t_[:, ds(half * T, T)], ps)
                kpT.append(t_)

            # ---------- Vp[s, 8 heads x (64 + ones)] = valsn @ Wv.T (bf16) --------
            vp = []
            for st in range(NS):
                t_ = main.tile([P, H, DH + 1], bf16, tag=f"vp{st}", name=f"vp{st}")
                nc.vector.memset(t_[:, :, DH : DH + 1], 1.0)
                ps = psmm.tile([P, D], f32, tag="mm")
                for dc in range(ND):
                    nc.tensor.matmul(
                        ps,
                        lhsT=vtr[dc][:, ts(st, P)],
                        rhs=wvT[dc][:],
                        start=(dc == 0), stop=(dc == ND - 1),
                    )
                nc.vector.tensor_copy(
                    t_[:, :, 0:DH], ps.rearrange("p (h e) -> p h e", h=H)
                )
                vp.append(t_)

            # ---------- qhT[e, t] = (Wq @ qT) / 8  (f32r) ----------
            qhT = []
            for e in range(ND):
                t_ = main.tile([P, T], f32r, tag=f"wvw{e}", name=f"qh{e}")
                ps = psmm.tile([P, T], f32, tag="mm")
                for dc in range(ND):
                    nc.tensor.matmul(
                        ps, lhsT=wqT[dc][:, ts(e, P)], rhs=qTr[dc][:],
                        start=(dc == 0), stop=(dc == ND - 1),
                    )
                nc.scalar.mul(t_[:], ps, 1.0 / np.sqrt(DH))
                qhT.append(t_)

            # ---------- transpose the mask to [s, t] (bf16 PE transposes) ---------
            mT = []
            for j in range(NS):
                tag = ["qry0", "qry1", "qry2", "wqp0", "mT4", "mT5", "mT6", "mT7"][j]
                mT.append(main.tile([P, T], bf16, tag=tag, name=f"mT{j}"))
            for tt in range(NT):
                for j in range(NS):
                    ps_t = psB.tile([P, P], bf16, tag="bc")
                    nc.tensor.transpose(ps_t, mask01[tt][:, ts(j, P)], ident)
                    nc.scalar.copy(mT[j][:, ts(tt, P)], ps_t)

            # ---------- masked attention over all S slots ----------
            # Both heads of an e-tile share one [128, 1024] att PSUM (2 banks):
            # even head in cols 0:T, odd head in cols T:2T — one exp + one
            # broadcast mask-mult covers both.
            # wkw slots chain: WkT -> ctxT
            ctxT = [
                main.tile([P, T], f32, tag=f"wkw{dt_i}", name=f"cx{dt_i}")
                for dt_i in range(ND)
            ]
            for et in range(ND):
                den_pair = scr2.tile([1, 2 * T], f32r, tag="den")
                ps_ctx_e = psA.tile([DH + 1, T], f32, tag="ctx", name="ctx_e")
                ps_ctx_o = psA.tile([DH + 1, T], f32, tag="ctx", name="ctx_o")
                for c in range(NS):
                    ps_att = psmm.tile([P, 2 * T], f32, tag="mm", name="att2")
                    for par in range(2):
                        nc.tensor.matmul(
                            ps_att[:, ds(par * T, T)],
                            lhsT=kpT[et][64 * par : 64 * par + DH, ts(c, P)],
                            rhs=qhT[et][64 * par : 64 * par + DH, :],
                            start=True, stop=True,
                        )
                    u = scr4.tile([P, 2 * T], bf16, tag="u")
                    nc.scalar.activation(u[:], ps_att, AF.Exp)
                    w = scr4.tile([P, 2 * T], bf16, tag="w")
                    nc.vector.tensor_tensor(
                        w[:].rearrange("p (x t) -> p x t", x=2),
                        u[:].rearrange("p (x t) -> p x t", x=2),
                        mT[c][:, None, :].to_broadcast([P, 2, T]),
                        OP.mult,
                    )
                    nc.tensor.matmul(
                        ps_ctx_e, lhsT=vp[c][:, 2 * et, :], rhs=w[:, 0:T],
                        start=(c == 0), stop=(c == NS - 1),
                    )
                    nc.tensor.matmul(
                        ps_ctx_o, lhsT=vp[c][:, 2 * et + 1, :], rhs=w[:, T : 2 * T],
                        start=(c == 0), stop=(c == NS - 1),
                    )
                for par, ps_ctx in ((0, ps_ctx_e), (1, ps_ctx_o)):
                    nc.scalar.copy(
                        ctxT[et][64 * par : 64 * par + DH, :].bitcast(f32r),
                        ps_ctx[0:DH, :],
                    )
                    nc.scalar.copy(
                        den_pair[0:1, ds(par * T, T)], ps_ctx[DH : DH + 1, :]
                    )
                # divide the head pair's ctx rows by their softmax denominators
                nc.vector.reciprocal(den_pair[:], den_pair[:].bitcast(f32))
                ps_rb = psB.tile([P, T], f32, tag="bc")
                nc.tensor.matmul(
                    ps_rb, lhsT=selA, rhs=den_pair[0:1, 0:T],
                    start=True, stop=False,
                )
                nc.tensor.matmul(
                    ps_rb, lhsT=selB, rhs=den_pair[0:1, T : 2 * T],
                    start=False, stop=True,
                )
                nc.vector.tensor_tensor(
                    ctxT[et][:].bitcast(f32r), ctxT[et][:], ps_rb, OP.mult
                )

            # ---------- oT[e, t] = Wo @ ctx.T  (f32r); wq slots -> oT ----------
            oT = []
            for e in range(ND):
                t_ = main.tile([P, T], f32, tag=f"wq{e}", name=f"o{e}")
                ps = psmm.tile([P, T], f32, tag="mm")
                for dc in range(ND):
                    nc.tensor.matmul(
                        ps, lhsT=woT[dc][:, ts(e, P)],
                        rhs=ctxT[dc][:].bitcast(f32r),
                        start=(dc == 0), stop=(dc == ND - 1),
                    )
                nc.scalar.copy(t_[:], ps)
                oT.append(t_)

            # ---------- LayerNorm over e (partitions), stats via ones-matmul -----
            ps_mu = psB.tile([1, T], f32, tag="row", name="psmu")
            ps_ms = psA.tile([1, T], f32, tag="ctx", name="psms")
            for dc in range(ND):
                sq = scr2.tile([P, T], f32, tag="sq")
                nc.scalar.square(sq, oT[dc][:])
                nc.tensor.matmul(
                    ps_mu, lhsT=ones_col, rhs=oT[dc][:],
                    start=(dc == 0), stop=(dc == ND - 1),
                )
                nc.tensor.matmul(
                    ps_ms, lhsT=ones_col, rhs=sq[:],
                    start=(dc == 0), stop=(dc == ND - 1),
                )
            mu_row = main.tile([1, T], f32, tag="mu", name="mu")
            ms_row = main.tile([1, T], f32, tag="ms", name="ms")
            nc.scalar.mul(mu_row[:], ps_mu, 1.0 / D)
            nc.scalar.mul(ms_row[:], ps_ms, 1.0 / D)
            var_row = main.tile([1, T], f32, tag="var", name="var")
            nc.vector.tensor_tensor(var_row[:], mu_row[:], mu_row[:], OP.mult)
            nc.vector.tensor_sub(var_row[:], ms_row[:], var_row[:])
            sd_row2 = main.tile([1, T], f32, tag="sd", name="sd2")
            nc.scalar.activation(sd_row2[:], var_row[:], AF.Sqrt, bias=eps_ln[:])
            rstd_row = main.tile([1, T], f32, tag="rstd", name="rstd")
            nc.vector.reciprocal(rstd_row[:], sd_row2[:])
            crow_r = main.tile([1, T], f32r, tag="mu2", name="crow_r")
            nc.vector.scalar_tensor_tensor(
                crow_r[:], mu_row[:], -1.0, rstd_row[:], op0=OP.mult, op1=OP.mult
            )
            rstd_r = main.tile([1, T], f32r, tag="ms2", name="rstd_r")
            nc.vector.tensor_copy(rstd_r[:], rstd_row[:])
            rstdB = main.tile([P, T], f32, tag="wqp1", name="rstdB")
            cB = main.tile([P, T], f32, tag="wqp2", name="cB")
            for row, dst in ((rstd_r, rstdB), (crow_r, cB)):
                ps_b = psB.tile([P, T], f32, tag="bc")
                nc.tensor.matmul(
                    ps_b, lhsT=ones_row_r, rhs=row[:], start=True, stop=True
                )
                nc.scalar.copy(dst[:], ps_b)
            nrm = []
            for dt_i in range(ND):
                nc.vector.tensor_tensor(oT[dt_i][:], oT[dt_i][:], rstdB[:], OP.mult)
                nc.vector.tensor_tensor(oT[dt_i][:], oT[dt_i][:], cB[:], OP.add)
                n_ = main.tile([P, T], f32r, tag=f"wkw{dt_i}", name=f"nrm{dt_i}")
                nc.vector.scalar_tensor_tensor(
                    n_[:], oT[dt_i][:], g_sb[:, dt_i : dt_i + 1],
                    b_sb[:, dt_i : dt_i + 1].to_broadcast([P, T]),
                    op0=OP.mult, op1=OP.add,
                )
                nrm.append(n_)

            # ---------- outT[q, t] = Wout @ normed.T + bout ----------
            for qt, (off, sz) in enumerate(QD_TILES):
                ps = psmm.tile([P, T], f32, tag="mm")
                for e in range(ND):
                    nc.tensor.matmul(
                        ps[:sz, :], lhsT=woutT[e][:, ds(off, sz)], rhs=nrm[e][:],
                        start=(e == 0), stop=(e == ND - 1),
                    )
                ot_sb = scr2.tile([P, T], f32, tag="ot")
                nc.scalar.add(ot_sb[:sz, :], ps[:sz, :], bout_sb[:sz, qt : qt + 1])
                nc.sync.dma_start(out_dram.ap()[ds(off, sz), :], ot_sb[:sz, :])

    nc.compile()
    return nc


def _prep_in_maps(inputs):
    def c(a):
        return np.ascontiguousarray(a, dtype=np.float32)

    q = np.asarray(inputs["query_states"], dtype=np.float32).reshape(B * N, QD)
    shared = {
        "WqpT": c(np.asarray(inputs["Wqp"]).T),
        "WqT": c(np.asarray(inputs["Wq"]).T),
        "WkT": c(np.asarray(inputs["Wk"]).T),
        "WvT": c(np.asarray(inputs["Wv"]).T),
        "WoT": c(np.asarray(inputs["Wo"]).T),
        "WoutT": c(np.asarray(inputs["Wout"]).T),
        "memkT": c(np.asarray(inputs["mem_keys"]).T),
        "memvT": c(np.asarray(inputs["mem_values"]).T),
        "ln_g": c(np.asarray(inputs["ln_g"])),
        "ln_b": c(np.asarray(inputs["ln_b"])),
        "bout": c(np.pad(np.asarray(inputs["bout"]), (0, 384 - QD))),
    }
    in_maps = []
    for core in range(NCORES):
        m = dict(shared)
        m["queryT"] = c(q[core * T : (core + 1) * T, :].T)
        in_maps.append(m)
    return in_maps


def kernel(**inputs) -> np.ndarray:
    if "nc" not in _CACHE:
        _CACHE["nc"] = _build_nc()
    nc = _CACHE["nc"]
    in_maps = _prep_in_maps(inputs)
    res = run_bass_kernel_spmd(nc, in_maps, core_ids=list(range(NCORES)))
    out = np.empty((B * N, QD), dtype=np.float32)
    for core in range(NCORES):
        out[core * T : (core + 1) * T, :] = res.results[core]["outT"].T
    return out.reshape(B, N, QD)


# revision 43
# speedup vs baseline: 1.1382x; 1.1382x over previous
"""GatedLTMMemory kernel for 8 Trainium2 NeuronCores.

Data-parallel over the 4096 flattened (B,N) tokens: 512 tokens per core.
Memory-slot tables and weights are replicated. The reference's per-selected-slot
projections (137 GFLOP) are replaced by projecting the slot tables once and
running a masked full-softmax over all S slots (exactly equivalent math).

Precision plan (fp32 matmuls run at 1/4 PE rate; float32r/bf16 at full rate):
  exact fp32 : selection path (q projection, slot norms, scores) — the top-32
               boundary gaps are ~1e-6 so this path cannot be rounded.
  float32r   : Kp/Vp/qh projections, attention logits, Wo/Wout epilogue
               (~1.6e-4 measured on HW).
  bf16       : softmax weights w = exp(att)*mask and the value table Vp
               (~2e-3; the denominators come from the same w so it cancels).

Emission order is chosen so the DVE top-k overlaps the PE Kp/Vp/qh
projections. SBUF pool tags are allocated statically, so dead tensors donate
their slots to later tensors (chains are noted inline). Host passes
weights/tables pre-transposed (layout prep only; no FLOPs moved to host).
"""

import numpy as np

import concourse.bacc as bacc
import concourse.mybir as mybir
import concourse.tile as tile
from concourse.bass import ds, ts
from concourse.bass_utils import run_bass_kernel_spmd
from concourse.masks import make_identity

B, N, QD, D, S, H, K = 4, 1024, 320, 512, 1024, 8, 32
DH = D // H
EPS = 1e-5
P = 128
T = 512                       # tokens per core
NCORES = 8
NT = T // P                   # 4 token tiles
ND = D // P                   # 4 contraction chunks over D
NS = S // P                   # 8 slot tiles
NEG = -1e30
QD_TILES = [(0, 128), (128, 128), (256, 64)]

f32 = mybir.dt.float32
f32r = mybir.dt.float32r
bf16 = mybir.dt.bfloat16
AF = mybir.ActivationFunctionType
OP = mybir.AluOpType

_CACHE: dict = {}


def _build_nc():
    nc = bacc.Bacc("TRN2", target_bir_lowering=False, debug=False)

    dr = {}

    def din(name, shape, dt_):
        dr[name] = nc.dram_tensor(name, shape, dt_, kind="ExternalInput")

    din("queryT", (QD, T), f32)
    din("WqpT", (QD, D), f32)
    din("WqT", (D, D), f32r)
    din("WkT", (D, D), f32r)
    din("WvT", (D, D), f32r)
    din("WoT", (D, D), f32r)
    din("WoutT", (D, QD), f32r)
    din("memkT", (D, S), f32)
    din("memvT", (D, S), f32)
    din("ln_g", (D,), f32)
    din("ln_b", (D,), f32)
    din("bout", (384,), f32)
    out_dram = nc.dram_tensor("outT", (QD, T), f32, kind="ExternalOutput")

    with tile.TileContext(nc) as tc:
        with (
            tc.tile_pool(name="const", bufs=1) as const,
            tc.tile_pool(name="main", bufs=1) as main,
            tc.tile_pool(name="scr2", bufs=2) as scr2,
            tc.tile_pool(name="scr4", bufs=4) as scr4,
            tc.tile_pool(name="psA", bufs=2, space="PSUM") as psA,
            tc.tile_pool(name="psB", bufs=1, space="PSUM") as psB,
            tc.tile_pool(name="psmm", bufs=3, space="PSUM") as psmm,
            nc.allow_low_precision(reason="validated f32r/bf16 paths"),
        ):
            # ---------- constants ----------
            ident = const.tile([P, P], bf16, tag="ident")
            make_identity(nc, ident)
            ones_col = const.tile([P, 1], f32, tag="ones_col")
            nc.vector.memset(ones_col, 1.0)
            ones_row = const.tile([1, P], f32, tag="ones_row")
            nc.vector.memset(ones_row, 1.0)
            # f32r half-ones rows for per-head-pair broadcast matmuls
            halfsel = const.tile([1, 2 * P], f32, tag="halfsel")
            nc.vector.memset(halfsel, 0.0)
            nc.vector.memset(halfsel[0:1, 64:192], 1.0)
            halfsel_r = const.tile([1, 2 * P], f32r, tag="halfsel_r")
            nc.scalar.copy(halfsel_r[:], halfsel[:])
            # halfsel layout: [0:64]=0, [64:192]=1, [192:256]=0
            ones_row_r = halfsel_r[0:1, 64:192]  # [1,128] all ones
            selA = halfsel_r[0:1, 128:256]       # [1,128]: ones x64, zeros x64
            selB = halfsel_r[0:1, 0:128]         # [1,128]: zeros x64, ones x64
            eps_tab = const.tile([P, 1], f32, tag="eps_tab")
            nc.vector.memset(eps_tab, 1e-12)
            eps_ln = const.tile([1, 1], f32, tag="eps_ln")
            nc.vector.memset(eps_ln, EPS)

            # ---------- weight loads ----------
            def load_rows(name, cols, row_tiles, tags, dt_):
                tiles = []
                for (off, sz), tag in zip(row_tiles, tags):
                    t_ = main.tile([sz, cols], dt_, tag=tag, name=f"ld_{tag}")
                    nc.sync.dma_start(t_[:], dr[name].ap()[ds(off, sz), :])
                    tiles.append(t_)
                return tiles

            d_rows = [(i * P, P) for i in range(ND)]
            qryT = load_rows("queryT", T, QD_TILES, ["qry0", "qry1", "qry2"], f32)
            wqpT = load_rows("WqpT", D, QD_TILES, ["wqp0", "wqp1", "wqp2"], f32)

            g_sb = const.tile([P, ND], f32, tag="g")
            nc.sync.dma_start(g_sb[:], dr["ln_g"].ap().rearrange("(o p) -> p o", p=P))
            b_sb = const.tile([P, ND], f32, tag="b")
            nc.sync.dma_start(b_sb[:], dr["ln_b"].ap().rearrange("(o p) -> p o", p=P))
            bout_sb = const.tile([P, 3], f32, tag="bout")
            nc.sync.dma_start(bout_sb[:], dr["bout"].ap().rearrange("(o p) -> p o", p=P))

            ktiles = load_rows("memkT", S, d_rows, [f"t14_{i}" for i in range(ND)], f32)

            from concourse import bass_isa

            # ---------- qT[d, t] = Wqp @ query.T (exact fp32; f32r copy for qh) ----
            # emitted first so the PE has work while the tables normalize
            qTr_tags = ["qry0", "qry1", "qry2", "wqp0"]
            qT = []
            for dt_i in range(ND):
                t_ = main.tile([P, T], f32, tag=f"qt{dt_i}", name=f"q{dt_i}")
                ps = psmm.tile([P, T], f32, tag="mm")
                for c in range(3):
                    nc.tensor.matmul(
                        ps, lhsT=wqpT[c][:, ts(dt_i, P)], rhs=qryT[c][:],
                        start=(c == 0), stop=(c == 2),
                    )
                nc.scalar.copy(t_[:], ps)
                qT.append(t_)
            qTr = []
            for dt_i in range(ND):
                tr_ = main.tile([P, T], f32r, tag=qTr_tags[dt_i], name=f"qr{dt_i}")
                nc.vector.tensor_copy(tr_[:], qT[dt_i][:])
                qTr.append(tr_)

            wqT = load_rows("WqT", D, d_rows, [f"wq{i}" for i in range(ND)], f32r)
            wkT = load_rows("WkT", D, d_rows, [f"wkw{i}" for i in range(ND)], f32r)
            vtiles = load_rows("memvT", S, d_rows, [f"t58_{i}" for i in range(ND)], f32)
            wvT = load_rows("WvT", D, d_rows, [f"wvw{i}" for i in range(ND)], f32r)
            woT = load_rows("WoT", D, d_rows, [f"wo{i}" for i in range(ND)], f32r)
            woutT = load_rows("WoutT", QD, d_rows, [f"wu{i}" for i in range(ND)], f32r)

            # ---------- slot tables: l2-normalize in transposed layout ----------
            # keys (on the scores critical path): PE ones-matmul for the
            # partition sum-of-squares. vals (off critical path): GPSIMD
            # partition_all_reduce, whose output is replicated so the rescale
            # needs no broadcast matmul.
            def normalize_keys(tiles):
                ps_halves = []
                for half in range(2):
                    if half == 0:
                        ps_ssq = psB.tile([1, T], f32, tag="row", name="ssq0")
                    else:
                        ps_ssq = psA.tile([1, T], f32, tag="ctx", name="ssq1")
                    for i in range(ND):
                        sq = scr2.tile([P, T], f32, tag="sq")
                        nc.scalar.square(sq, tiles[i][:, ds(half * T, T)])
                        nc.tensor.matmul(
                            ps_ssq, lhsT=ones_col, rhs=sq,
                            start=(i == 0), stop=(i == ND - 1),
                        )
                    ps_halves.append(ps_ssq)
                sd_row = main.tile([1, S], f32, tag="sdrow", name="sdr")
                for half in range(2):
                    nc.scalar.activation(
                        sd_row[:, ds(half * T, T)], ps_halves[half], AF.Sqrt,
                        bias=eps_tab[0:1, :],
                    )
                rsq_row = main.tile([1, S], f32, tag="rsqrow", name="rsq")
                nc.vector.reciprocal(rsq_row, sd_row)
                rsqB = main.tile([P, S], f32, tag="rsqB", name="rsqB")
                for half in range(2):
                    ps_b = psA.tile([P, T], f32, tag="bc")
                    nc.tensor.matmul(
                        ps_b, lhsT=ones_row, rhs=rsq_row[:, ds(half * T, T)],
                        start=True, stop=True,
                    )
                    nc.scalar.copy(rsqB[:, ds(half * T, T)], ps_b)
                for i in range(ND):
                    nc.vector.tensor_tensor(tiles[i][:], tiles[i][:], rsqB[:], OP.mult)
                return tiles

            def normalize_vals(tiles):
                sqsum = main.tile([P, S], f32, tag="rsqrow", name="sqs")
                for i in range(ND):
                    sq = main.tile([P, S], f32, tag=f"wk{i}", name=f"vsq{i}")
                    nc.scalar.square(sq[:], tiles[i][:])
                    if i == 0:
                        nc.gpsimd.tensor_copy(sqsum[:], sq[:])
                    else:
                        nc.gpsimd.tensor_tensor(sqsum[:], sqsum[:], sq[:], OP.add)
                rsq_full = main.tile([P, S], f32, tag="rsqB", name="rsqf")
                nc.gpsimd.partition_all_reduce(
                    rsq_full[:], sqsum[:], channels=P, reduce_op=bass_isa.ReduceOp.add
                )
                nc.scalar.activation(sqsum[:], rsq_full[:], AF.Sqrt, bias=eps_tab[:])
                nc.vector.reciprocal(rsq_full[:], sqsum[:])
                for i in range(ND):
                    nc.gpsimd.tensor_tensor(
                        tiles[i][:], tiles[i][:], rsq_full[:], OP.mult
                    )
                return tiles

            # keys; t14 slots chain: keysnT -> mask01
            keysnT = normalize_keys(ktiles)
            # rounded copy of keysnT for the f32r KpT matmul (scores keep fp32)
            ktr = []
            for i in range(ND):
                t_ = main.tile([P, S], f32r, tag=f"ktr{i}", name=f"ktr{i}")
                nc.vector.tensor_copy(t_[:], keysnT[i][:])
                ktr.append(t_)
            # vals; t58 slots chain: valsnT -> scores; wk: vals-sq -> topk scratch
            valsnT = normalize_vals(vtiles)
            vtr_tags = ["sdrow", "rsqrow", "rsqB", "vtr3"]
            vtr = []
            for i in range(ND):
                t_ = main.tile([P, S], f32r, tag=vtr_tags[i], name=f"vtr{i}")
                nc.vector.tensor_copy(t_[:], valsnT[i][:])
                vtr.append(t_)

            # ---------- scores[t, s] = q @ keysn.T (exact fp32), then top-32 ------
            sc = []
            for tt in range(NT):
                t_ = main.tile([P, S], f32, tag=f"t58_{tt}", name=f"sc{tt}")
                for half in range(2):
                    ps = psmm.tile([P, T], f32, tag="mm")
                    for dc in range(ND):
                        nc.tensor.matmul(
                            ps,
                            lhsT=qT[dc][:, ts(tt, P)],
                            rhs=keysnT[dc][:, ds(half * T, T)],
                            start=(dc == 0), stop=(dc == ND - 1),
                        )
                    nc.scalar.copy(t_[:, ds(half * T, T)], ps)
                sc.append(t_)

            # top-32 threshold per token row (4 rounds of max8), then bf16 mask
            mask01 = []
            for tt in range(NT):
                work = main.tile([P, S], f32, tag=f"wk{tt}", name=f"wk{tt}")
                cur = sc[tt]
                for r in range(4):
                    mx = main.tile([P, 8], f32, tag=f"mx{tt}_{r}", name=f"mx{tt}_{r}")
                    nc.vector.max(out=mx[:], in_=cur[:])
                    if r < 3:
                        nc.vector.match_replace(
                            out=work[:], in_to_replace=mx[:], in_values=cur[:],
                            imm_value=NEG,
                        )
                        cur = work
                m_ = main.tile([P, S], bf16, tag=f"t14_{tt}", name=f"mk{tt}")
                nc.vector.tensor_scalar(
                    m_[:], sc[tt][:], mx[:, 7:8], None, op0=OP.is_ge
                )
                mask01.append(m_)

            # ---------- KpT[e, s] = Wk @ keysn.T  (f32r) ----------
            kpT = []
            for e in range(ND):
                t_ = main.tile([P, S], f32r, tag=f"kp{e}", name=f"kp{e}")
                for half in range(2):
                    ps = psmm.tile([P, T], f32, tag="mm")
                    for dc in range(ND):
                        nc.tensor.matmul(
                            ps,
                            lhsT=wkT[dc][:, ts(e, P)],
                            rhs=ktr[dc][:, ds(half * T, T)],
                            start=(dc == 0), stop=(dc == ND - 1),
                        )
                    nc.scalar.copy(t_[:, ds(half * T, T)], ps)
                kpT.append(t_)

            # ---------- Vp[s, 8 heads x (64 + ones)] = valsn @ Wv.T (bf16) --------
            vp = []
            for st in range(NS):
                t_ = main.tile([P, H, DH + 1], bf16, tag=f"vp{st}", name=f"vp{st}")
                nc.vector.memset(t_[:, :, DH : DH + 1], 1.0)
                ps = psmm.tile([P, D], f32, tag="mm")
                for dc in range(ND):
                    nc.tensor.matmul(
                        ps,
                        lhsT=vtr[dc][:, ts(st, P)],
                        rhs=wvT[dc][:],
                        start=(dc == 0), stop=(dc == ND - 1),
                    )
                nc.vector.tensor_copy(
                    t_[:, :, 0:DH], ps.rearrange("p (h e) -> p h e", h=H)
                )
                vp.append(t_)

            # ---------- qhT[e, t] = (Wq @ qT) / 8  (f32r) ----------
            qhT = []
            for e in range(ND):
                t_ = main.tile([P, T], f32r, tag=f"wvw{e}", name=f"qh{e}")
                ps = psmm.tile([P, T], f32, tag="mm")
                for dc in range(ND):
                    nc.tensor.matmul(
                        ps, lhsT=wqT[dc][:, ts(e, P)], rhs=qTr[dc][:],
                        start=(dc == 0), stop=(dc == ND - 1),
                    )
                nc.scalar.mul(t_[:], ps, 1.0 / np.sqrt(DH))
                qhT.append(t_)

            # ---------- transpose the mask to [s, t] (bf16 PE transposes) ---------
            mT = []
            for j in range(NS):
                tag = ["qry0", "qry1", "qry2", "wqp0", "mT4", "mT5", "mT6", "mT7"][j]
                mT.append(main.tile([P, T], bf16, tag=tag, name=f"mT{j}"))
            for tt in range(NT):
                for j in range(NS):
                    ps_t = psA.tile([P, P], bf16, tag="bc")
                    nc.tensor.transpose(ps_t, mask01[tt][:, ts(j, P)], ident)
                    nc.scalar.copy(mT[j][:, ts(tt, P)], ps_t)

            # ---------- masked attention over all S slots ----------
            # Both heads of an e-tile share one [128, 1024] att PSUM (2 banks):
            # even head in cols 0:T, odd head in cols T:2T — one exp + one
            # broadcast mask-mult covers both.
            # wkw slots chain: WkT -> ctxT
            ctxT = [
                main.tile([P, T], f32, tag=f"wkw{dt_i}", name=f"cx{dt_i}")
                for dt_i in range(ND)
            ]
            for h in range(H):
                et, ro = h // 2, (h % 2) * 64
                if h % 2 == 0:
                    den_pair = scr2.tile([1, 2 * T], f32r, tag="den")
                ps_ctx = psA.tile([DH + 1, T], f32, tag="ctx")
                for c in range(NS):
                    ps_att = psmm.tile([P, T], f32, tag="mm")
                    nc.tensor.matmul(
                        ps_att,
                        lhsT=kpT[et][ro : ro + DH, ts(c, P)],
                        rhs=qhT[et][ro : ro + DH, :],
                        start=True, stop=True,
                    )
                    u = scr4.tile([P, T], bf16, tag="u")
                    nc.scalar.activation(u[:], ps_att, AF.Exp)
                    w = scr4.tile([P, T], bf16, tag="w")
                    nc.vector.tensor_tensor(w[:], u[:], mT[c][:], OP.mult)
                    nc.tensor.matmul(
                        ps_ctx, lhsT=vp[c][:, h, :], rhs=w[:],
                        start=(c == 0), stop=(c == NS - 1),
                    )
                nc.vector.tensor_copy(
                    ctxT[et][ro : ro + DH, :].bitcast(f32r), ps_ctx[0:DH, :]
                )
                nc.vector.tensor_copy(
                    den_pair[0:1, ds((h % 2) * T, T)], ps_ctx[DH : DH + 1, :]
                )
                if h % 2 == 1:
                    # divide the head pair's ctx rows by their softmax denominators
                    nc.vector.reciprocal(den_pair[:], den_pair[:].bitcast(f32))
                    ps_rb = psA.tile([P, T], f32, tag="bc")
                    nc.tensor.matmul(
                        ps_rb, lhsT=selA, rhs=den_pair[0:1, 0:T],
                        start=True, stop=False,
                    )
                    nc.tensor.matmul(
                        ps_rb, lhsT=selB, rhs=den_pair[0:1, T : 2 * T],
                        start=False, stop=True,
                    )
                    nc.vector.tensor_tensor(
                        ctxT[et][:].bitcast(f32r), ctxT[et][:], ps_rb, OP.mult
                    )

            # ---------- oT[e, t] = Wo @ ctx.T  (f32r); wq slots -> oT ----------
            oT = []
            for e in range(ND):
                t_ = main.tile([P, T], f32, tag=f"wq{e}", name=f"o{e}")
                ps = psmm.tile([P, T], f32, tag="mm")
                for dc in range(ND):
                    nc.tensor.matmul(
                        ps, lhsT=woT[dc][:, ts(e, P)],
                        rhs=ctxT[dc][:].bitcast(f32r),
                        start=(dc == 0), stop=(dc == ND - 1),
                    )
                nc.scalar.copy(t_[:], ps)
                oT.append(t_)

            # ---------- LayerNorm over e (partitions), stats via ones-matmul -----
            ps_mu = psB.tile([1, T], f32, tag="row", name="psmu")
            ps_ms = psA.tile([1, T], f32, tag="ctx", name="psms")
            for dc in range(ND):
                sq = scr2.tile([P, T], f32, tag="sq")
                nc.scalar.square(sq, oT[dc][:])
                nc.tensor.matmul(
                    ps_mu, lhsT=ones_col, rhs=oT[dc][:],
                    start=(dc == 0), stop=(dc == ND - 1),
                )
                nc.tensor.matmul(
                    ps_ms, lhsT=ones_col, rhs=sq[:],
                    start=(dc == 0), stop=(dc == ND - 1),
                )
            mu_row = main.tile([1, T], f32, tag="mu", name="mu")
            ms_row = main.tile([1, T], f32, tag="ms", name="ms")
            nc.scalar.mul(mu_row[:], ps_mu, 1.0 / D)
            nc.scalar.mul(ms_row[:], ps_ms, 1.0 / D)
            var_row = main.tile([1, T], f32, tag="var", name="var")
            nc.vector.tensor_tensor(var_row[:], mu_row[:], mu_row[:], OP.mult)
            nc.vector.tensor_sub(var_row[:], ms_row[:], var_row[:])
            sd_row2 = main.tile([1, T], f32, tag="sd", name="sd2")
            nc.scalar.activation(sd_row2[:], var_row[:], AF.Sqrt, bias=eps_ln[:])
            rstd_row = main.tile([1, T], f32, tag="rstd", name="rstd")
            nc.vector.reciprocal(rstd_row[:], sd_row2[:])
            crow_r = main.tile([1, T], f32r, tag="mu2", name="crow_r")
            nc.vector.scalar_tensor_tensor(
                crow_r[:], mu_row[:], -1.0, rstd_row[:], op0=OP.mult, op1=OP.mult
            )
            rstd_r = main.tile([1, T], f32r, tag="ms2", name="rstd_r")
            nc.vector.tensor_copy(rstd_r[:], rstd_row[:])
            rstdB = main.tile([P, T], f32, tag="wqp1", name="rstdB")
            cB = main.tile([P, T], f32, tag="wqp2", name="cB")
            for row, dst in ((rstd_r, rstdB), (crow_r, cB)):
                ps_b = psA.tile([P, T], f32, tag="bc")
                nc.tensor.matmul(
                    ps_b, lhsT=ones_row_r, rhs=row[:], start=True, stop=True
                )
                nc.scalar.copy(dst[:], ps_b)
            nrm = []
            for dt_i in range(ND):
                nc.vector.tensor_tensor(oT[dt_i][:], oT[dt_i][:], rstdB[:], OP.mult)
                nc.vector.tensor_tensor(oT[dt_i][:], oT[dt_i][:], cB[:], OP.add)
                n_ = main.tile([P, T], f32r, tag=f"wkw{dt_i}", name=f"nrm{dt_i}")
                nc.vector.scalar_tensor_tensor(
                    n_[:], oT[dt_i][:], g_sb[:, dt_i : dt_i + 1],
                    b_sb[:, dt_i : dt_i + 1].to_broadcast([P, T]),
                    op0=OP.mult, op1=OP.add,
                )
                nrm.append(n_)

            # ---------- outT[q, t] = Wout @ normed.T + bout ----------
            for qt, (off, sz) in enumerate(QD_TILES):
                ps = psmm.tile([P, T], f32, tag="mm")
                for e in range(ND):
                    nc.tensor.matmul(
                        ps[:sz, :], lhsT=woutT[e][:, ds(off, sz)], rhs=nrm[e][:],
                        start=(e == 0), stop=(e == ND - 1),
                    )
                ot_sb = scr2.tile([P, T], f32, tag="ot")
                nc.scalar.add(ot_sb[:sz, :], ps[:sz, :], bout_sb[:sz, qt : qt + 1])
                nc.sync.dma_start(out_dram.ap()[ds(off, sz), :], ot_sb[:sz, :])

    nc.compile()
    return nc


def _prep_in_maps(inputs):
    def c(a):
        return np.ascontiguousarray(a, dtype=np.float32)

    q = np.asarray(inputs["query_states"], dtype=np.float32).reshape(B * N, QD)
    shared = {
        "WqpT": c(np.asarray(inputs["Wqp"]).T),
        "WqT": c(np.asarray(inputs["Wq"]).T),
        "WkT": c(np.asarray(inputs["Wk"]).T),
        "WvT": c(np.asarray(inputs["Wv"]).T),
        "WoT": c(np.asarray(inputs["Wo"]).T),
        "WoutT": c(np.asarray(inputs["Wout"]).T),
        "memkT": c(np.asarray(inputs["mem_keys"]).T),
        "memvT": c(np.asarray(inputs["mem_values"]).T),
        "ln_g": c(np.asarray(inputs["ln_g"])),
        "ln_b": c(np.asarray(inputs["ln_b"])),
        "bout": c(np.pad(np.asarray(inputs["bout"]), (0, 384 - QD))),
    }
    in_maps = []
    for core in range(NCORES):
        m = dict(shared)
        m["queryT"] = c(q[core * T : (core + 1) * T, :].T)
        in_maps.append(m)
    return in_maps


def kernel(**inputs) -> np.ndarray:
    if "nc" not in _CACHE:
        _CACHE["nc"] = _build_nc()
    nc = _CACHE["nc"]
    in_maps = _prep_in_maps(inputs)
    res = run_bass_kernel_spmd(nc, in_maps, core_ids=list(range(NCORES)))
    out = np.empty((B * N, QD), dtype=np.float32)
    for core in range(NCORES):
        out[core * T : (core + 1) * T, :] = res.results[core]["outT"].T
    return out.reshape(B, N, QD)
